# revision 1
# baseline (speedup 1.0000x reference)
"""Multi-head attention forward kernel for Trainium2 (8 NeuronCores).

Problem: B=2, N=2048, C=1024, H=16 heads, head_dim=64.
    q = x @ Wq.T + bq  (same for k, v)
    out = softmax(q k^T / sqrt(C)) v       (per head), re-merged to [B, N, C]

Sharding: core = (batch b, head-group g): b = core // 4, g = core % 4.
Each core computes 4 heads of one batch element. No collectives needed --
outputs are disjoint; host gathers and finishes with a cheap epilogue
(normalize by the row-sums and transpose).

Per-core design (measured ~190us: ACT-exp-bound at ~144us busy, PE ~155us,
DVE ~116us):
  - x/W matmul operands bf16 (host-converted); P/V fp16; PSUM fp32.
  - QT/KT [256, N] head-major (row-packed K=64 QK matmuls for both heads of
    a pair run concurrently on the PE); V [N, 4, 64] natural.
  - S^T chunk [128 keys, 2 heads, 512 q] fp32 PSUM (one bank per head);
    one ACT exp op covers both heads' chunks -> 128 exp ops total.
  - PV col-packed: both heads' O^T accumulate into one [128, 512] fp32 PSUM
    tile (head h at partitions h*64..h*64+63, tile_position col packing) --
    two concurrent M=64 matmuls per key chunk.
  - Softmax denominators: DVE accumulates P^T chunks into two fp16 parity
    accumulators (fast 2-byte DVE mode); ones-vector matmuls reduce over
    the 128 key partitions into PSUM partitions {0, 32} (tile_position).
  - Projection blocks are emitted inside the attention loops at the latest
    dependency-legal spot, so they fill PE idle under the ACT-bound exp
    stream instead of delaying it (emission order = scheduler priority).
  - Normalization + final transpose happen on the host (cheap epilogue).
Outputs: out_o [2, 128, N] (pair, head-major O^T rows, queries),
         out_s [2, 2, N]   (pair, head, query sums).
"""

import os
import sys

import ml_dtypes
import numpy as np

for _p in ("/opt/trn_rl_repo",):
    if _p not in sys.path:
        sys.path.insert(0, _p)

import concourse.bass as bass  # noqa: E402
import concourse.tile as tile  # noqa: E402
from concourse import bacc, mybir  # noqa: E402
from concourse.bass_utils import run_bass_kernel_spmd  # noqa: E402

N = 2048  # sequence length
C = 1024  # model dim
D = 64  # head dim
NH = 4  # heads per core
HD = NH * D  # 256 output channels per core
NCORES = 8
KB = N // 128  # 16 key chunks of 128
QB = N // 512  # 4 query blocks of 512
KC = C // 128  # 8 contraction chunks for projections
SCALE = 1.0 / 32.0  # 1 / sqrt(C)

F32 = mybir.dt.float32
BF16 = mybir.dt.bfloat16
FP16 = mybir.dt.float16


def build_kernel(tc, xt, wqt, wkt, wvt, bq, bk, bv, out_o, out_s):
    nc = tc.nc
    Exp = mybir.ActivationFunctionType.Exp

    with (
        tc.tile_pool(name="res", bufs=1) as res,
        tc.tile_pool(name="ppsum", bufs=2, space="PSUM") as ppsum,
        tc.tile_pool(name="stp", bufs=2, space="PSUM") as stp,
        tc.tile_pool(name="opp", bufs=1, space="PSUM") as opp,
        tc.tile_pool(name="sup", bufs=1, space="PSUM") as sup,
        tc.tile_pool(name="ptp", bufs=16) as ptp,
        tc.tile_pool(name="otp", bufs=2) as otp,
        tc.tile_pool(name="ssp", bufs=2) as ssp,
    ):
        # ---- resident SBUF tensors ----
        wq_all = res.tile([128, KC, HD], BF16, tag="wq", name="wq")
        wk_all = res.tile([128, KC, HD], BF16, tag="wk", name="wk")
        wv_all = res.tile([128, KC, HD], BF16, tag="wv", name="wv")
        xt_sb = [res.tile([128, N], BF16, tag=f"xt{k}", name=f"xt{k}") for k in range(KC)]
        wq_sb = [wq_all[:, k, :] for k in range(KC)]
        wk_sb = [wk_all[:, k, :] for k in range(KC)]
        wv_sb = [wv_all[:, k, :] for k in range(KC)]
        qt_sb = [res.tile([128, N], BF16, tag=f"qt{m}", name=f"qt{m}") for m in range(2)]
        kt_sb = [res.tile([128, N], BF16, tag=f"kt{m}", name=f"kt{m}") for m in range(2)]
        v_sb = [res.tile([128, NH, D], FP16, tag=f"v{kb}", name=f"v{kb}") for kb in range(KB)]
        bq_sb = [res.tile([128, 1], F32, tag=f"bq{m}", name=f"bq{m}") for m in range(2)]
        bk_sb = [res.tile([128, 1], F32, tag=f"bk{m}", name=f"bk{m}") for m in range(2)]
        bv_sb = res.tile([128, HD], F32, tag="bv", name="bv")
        ones_sb = res.tile([128, 1], FP16, tag="ones", name="ones")
        warm_sb = res.tile([1, 2], F32, tag="warm", name="warm")

        # ---- input DMAs: weights for the first projections, then x chunks ----
        nc.sync.dma_start(out=wq_all[:], in_=wqt.rearrange("(k p) n -> p k n", p=128))
        nc.sync.dma_start(out=wk_all[:], in_=wkt.rearrange("(k p) n -> p k n", p=128))
        for k in range(KC):
            nc.sync.dma_start(out=xt_sb[k][:], in_=xt[k * 128 : (k + 1) * 128, :])
        nc.sync.dma_start(out=wv_all[:], in_=wvt.rearrange("(k p) n -> p k n", p=128))
        for m in range(2):
            sl = slice(m * 128, (m + 1) * 128)
            nc.sync.dma_start(out=bq_sb[m][:], in_=bq[sl])
            nc.sync.dma_start(out=bk_sb[m][:], in_=bk[sl])
        bv_bcast = bass.AP(tensor=bv.tensor, offset=bv.offset, ap=[[0, 128]] + list(bv.ap))
        nc.sync.dma_start(out=bv_sb[:], in_=bv_bcast)
        nc.vector.memset(ones_sb[:], 1.0)
        # warm up the ACT exp table while DMAs land
        nc.vector.memset(warm_sb[:], 0.0)
        nc.scalar.activation(out=warm_sb[:, 0:1], in_=warm_sb[:, 1:2], func=Exp)

        def proj_qk_block(which, m, nb):
            w_sb = wq_sb if which == "q" else wk_sb
            b_sb = (bq_sb if which == "q" else bk_sb)[m]
            t_sb = (qt_sb if which == "q" else kt_sb)[m]
            nsl = slice(nb * 512, (nb + 1) * 512)
            ps = ppsum.tile([128, 512], F32, tag="qkps", name="qkps")
            for k in range(KC):
                nc.tensor.matmul(
                    out=ps[:],
                    lhsT=w_sb[k][:, m * 128 : (m + 1) * 128],
                    rhs=xt_sb[k][:, nsl],
                    start=(k == 0),
                    stop=(k == KC - 1),
                )
            nc.vector.tensor_scalar_add(out=t_sb[:, nsl], in0=ps[:], scalar1=b_sb[:])

        def proj_v_block(kb):
            vps = ppsum.tile([128, HD], F32, tag="qkps", name="vps")
            for k in range(KC):
                nc.tensor.matmul(
                    out=vps[:],
                    lhsT=xt_sb[k][:, kb * 128 : (kb + 1) * 128],
                    rhs=wv_sb[k][:],
                    start=(k == 0),
                    stop=(k == KC - 1),
                )
            nc.vector.tensor_add(
                out=v_sb[kb][:],
                in0=vps[:].rearrange("p (h d) -> p h d", h=NH),
                in1=bv_sb[:].rearrange("p (h d) -> p h d", h=NH),
            )

        def attn(p, pre_pv_hook=None, post_exp_hook=None):
            for qb in range(QB):
                qsl = slice(qb * 512, (qb + 1) * 512)
                # both heads' O^T col-packed: head h at partitions h*64..
                o_ps = opp.tile([128, 512], F32, tag="o", name="o")
                # running sums of P^T chunks (softmax denominators): two
                # fp16 parity accumulators keep the DVE in its fast 2-byte
                # mode and halve the accumulation depth.
                ssum = [
                    ssp.tile([128, 2, 512], FP16, tag=f"ssum{j}", name=f"ssum{j}")
                    for j in range(2)
                ]

                def emit_pv(args):
                    kb, pt = args
                    for h in range(2):
                        nc.tensor.matmul(
                            out=o_ps[h * D : (h + 1) * D, :],
                            lhsT=v_sb[kb][:, 2 * p + h, :],
                            rhs=pt[:, h, :],
                            start=(kb == 0),
                            stop=(kb == KB - 1),
                            tile_position=(0, h * D),
                            skip_group_check=True,
                        )
                    sj = ssum[kb % 2]
                    if kb < 2:
                        nc.vector.tensor_copy(out=sj[:], in_=pt[:])
                    else:
                        nc.vector.tensor_add(out=sj[:], in0=sj[:], in1=pt[:])

                # PV + ssum are emitted one kb behind their exp so the
                # in-order PE never sits on the o-psum wait before issuing
                # the next QK pair (which would stall the ACT exp pipeline).
                prev = None
                for kb in range(KB):
                    if pre_pv_hook is not None:
                        pre_pv_hook(qb, kb)
                    ksl = slice(kb * 128, (kb + 1) * 128)
                    # st layout [128 keys, head, 512 q] fp32: head h
                    # occupies its own PSUM bank -> the two concurrently-
                    # drained row-packed matmuls hit different banks.
                    st = stp.tile([128, 2, 512], F32, tag="st", name="st")
                    for h in range(2):
                        hsl = slice(h * D, (h + 1) * D)
                        nc.tensor.matmul(
                            out=st[:, h, :],
                            lhsT=kt_sb[p][hsl, ksl],
                            rhs=qt_sb[p][hsl, qsl],
                            start=True,
                            stop=True,
                        )
                    pt = ptp.tile([128, 2, 512], FP16, tag="pt", name="pt")
                    nc.scalar.activation(out=pt[:], in_=st[:], func=Exp, scale=SCALE)
                    if post_exp_hook is not None:
                        post_exp_hook(qb, kb)
                    if prev is not None:
                        emit_pv(prev)
                    prev = (kb, pt)
                emit_pv(prev)

                # partition-reduce the running sums with ones-vector
                # matmuls (both parity accumulators accumulate into the same
                # PSUM row); head h lands at PSUM partition 32*h.
                s_ps = sup.tile([33, 512], F32, tag="sps", name="sps")
                for h in range(2):
                    for j in range(2):
                        nc.tensor.matmul(
                            out=s_ps[32 * h : 32 * h + 1, :],
                            lhsT=ones_sb[:],
                            rhs=ssum[j][:, h, :],
                            start=(j == 0),
                            stop=(j == 1),
                            tile_position=(0, 32 * h),
                            skip_group_check=True,
                        )
                ss = otp.tile([33, 512], F32, tag="ss", name="ss")
                for h in range(2):
                    nc.vector.tensor_copy(
                        out=ss[32 * h : 32 * h + 1, :],
                        in_=s_ps[32 * h : 32 * h + 1, :],
                    )
                ss_view = bass.AP(
                    tensor=ss.tensor, offset=ss.offset,
                    ap=[[32 * ss.ap[0][0], 2]] + list(ss.ap[1:]),
                )
                nc.sync.dma_start(out=out_s[p, :, qsl], in_=ss_view)
                ot = otp.tile([128, 512], F32, tag="ot", name="ot")
                nc.vector.tensor_copy(out=ot[:], in_=o_ps[:])
                nc.sync.dma_start(out=out_o[p, :, qsl], in_=ot[:])

        # ---- emission order doubles as scheduler priority, and dependency
        # tracking follows emission order -- producers must precede their
        # consumers.  Q/K pair 0 first (gates the first exp), V projection
        # interleaved per-kb into attention qb0 (each v tile lands just
        # before the PV that consumes it; the pt pool decouples the ACT exp
        # stream from the lagging PV chain), Q/K pair 1 as PE filler inside
        # pair-0's ACT-bound window.
        def proj_qk_first():
            qps = ppsum.tile([128, 512], F32, tag="qkps", name="qkps")
            kps = ppsum.tile([128, 512], F32, tag="qkps", name="qkps")
            for k in range(KC):
                for w_sb, ps in ((wq_sb, qps), (wk_sb, kps)):
                    nc.tensor.matmul(
                        out=ps[:],
                        lhsT=w_sb[k][:, 0:128],
                        rhs=xt_sb[k][:, 0:512],
                        start=(k == 0),
                        stop=(k == KC - 1),
                    )
            nc.vector.tensor_scalar_add(out=qt_sb[0][:, 0:512], in0=qps[:], scalar1=bq_sb[0][:])
            nc.vector.tensor_scalar_add(out=kt_sb[0][:, 0:512], in0=kps[:], scalar1=bk_sb[0][:])

        # Filler projection blocks are interleaved into both attention
        # pairs' ACT-bound windows, each at the latest iteration that still
        # precedes (in emission = dependency order) its first consumer, so
        # exps are never gated behind unrelated projection matmuls and the
        # filler spreads over the whole kernel's PE idle time.
        def pair0_hook(qb, kb):
            if qb == 0:
                proj_v_block(kb)

        def pair0_post(qb, kb):
            if qb == 0:
                if kb == 1:
                    proj_qk_block("k", 0, 1)
                elif kb == 5:
                    proj_qk_block("k", 0, 2)
                elif kb == 9:
                    proj_qk_block("k", 0, 3)
                elif kb == 13:
                    proj_qk_block("q", 0, 1)
            elif qb == 1:
                if kb == 1:
                    proj_qk_block("q", 0, 2)
                elif kb == 9:
                    proj_qk_block("q", 0, 3)
            elif qb == 3:
                if kb == 2:
                    proj_qk_block("k", 1, 0)
                elif kb == 6:
                    proj_qk_block("q", 1, 0)

        def pair1_post(qb, kb):
            if qb == 0:
                if kb == 1:
                    proj_qk_block("k", 1, 1)
                elif kb == 5:
                    proj_qk_block("k", 1, 2)
                elif kb == 9:
                    proj_qk_block("k", 1, 3)
                elif kb == 13:
                    proj_qk_block("q", 1, 1)
            elif qb == 1:
                if kb == 1:
                    proj_qk_block("q", 1, 2)
                elif kb == 9:
                    proj_qk_block("q", 1, 3)

        proj_qk_first()
        attn(0, pre_pv_hook=pair0_hook, post_exp_hook=pair0_post)
        attn(1, post_exp_hook=pair1_post)


def build_nc():
    nc = bacc.Bacc(
        "TRN2",
        target_bir_lowering=False,
        debug=False,
        num_devices=NCORES,
        enable_partition_id=False,
    )
    xt = nc.dram_tensor("xt", [C, N], BF16, kind="ExternalInput").ap()
    wqt = nc.dram_tensor("wqt", [C, HD], BF16, kind="ExternalInput").ap()
    wkt = nc.dram_tensor("wkt", [C, HD], BF16, kind="ExternalInput").ap()
    wvt = nc.dram_tensor("wvt", [C, HD], BF16, kind="ExternalInput").ap()
    bq = nc.dram_tensor("bq", [HD], F32, kind="ExternalInput").ap()
    bk = nc.dram_tensor("bk", [HD], F32, kind="ExternalInput").ap()
    bv = nc.dram_tensor("bv", [HD], F32, kind="ExternalInput").ap()
    out_o = nc.dram_tensor("out_o", [2, 128, N], F32, kind="ExternalOutput").ap()
    out_s = nc.dram_tensor("out_s", [2, 2, N], F32, kind="ExternalOutput").ap()

    with tile.TileContext(nc) as tc:
        build_kernel(tc, xt, wqt, wkt, wvt, bq, bk, bv, out_o, out_s)
    nc.compile()
    return nc


def shard_inputs(inputs):
    x = np.asarray(inputs["x"], np.float32)
    in_maps = []
    for core in range(NCORES):
        b, g = core // 4, core % 4
        sl = slice(g * HD, (g + 1) * HD)
        in_maps.append(
            {
                "xt": np.ascontiguousarray(x[b].T).astype(ml_dtypes.bfloat16),
                "wqt": np.ascontiguousarray(np.asarray(inputs["Wq"], np.float32)[sl, :].T).astype(ml_dtypes.bfloat16),
                "wkt": np.ascontiguousarray(np.asarray(inputs["Wk"], np.float32)[sl, :].T).astype(ml_dtypes.bfloat16),
                "wvt": np.ascontiguousarray(np.asarray(inputs["Wv"], np.float32)[sl, :].T).astype(ml_dtypes.bfloat16),
                "bq": np.ascontiguousarray(np.asarray(inputs["bq"], np.float32)[sl]),
                "bk": np.ascontiguousarray(np.asarray(inputs["bk"], np.float32)[sl]),
                "bv": np.ascontiguousarray(np.asarray(inputs["bv"], np.float32)[sl]),
            }
        )
    return in_maps


def assemble(results, B=2):
    out = np.zeros((B, N, C), np.float32)
    for core in range(NCORES):
        b, g = core // 4, core % 4
        oo = np.asarray(results[core]["out_o"], np.float32)  # [2, 128, N]
        os_ = np.asarray(results[core]["out_s"], np.float32)  # [2, 2, N]
        o = oo.reshape(2, 2, D, N)  # [pair, head, d, n]
        on = o / os_[:, :, None, :]
        # [pair, head, d, n] -> [n, pair*2*D + head*D + d]
        out[b, :, g * HD : (g + 1) * HD] = (
            on.transpose(3, 0, 1, 2).reshape(N, HD)
        )
    return out


_NC_CACHE = None


def _get_nc():
    global _NC_CACHE
    if _NC_CACHE is None:
        _NC_CACHE = build_nc()
    return _NC_CACHE


def kernel(**inputs):
    nc = _get_nc()
    in_maps = shard_inputs(inputs)
    res = run_bass_kernel_spmd(
        nc,
        in_maps,
        core_ids=list(range(NCORES)),
        trace=bool(int(os.environ.get("KERNEL_TRACE", "0"))),
    )
    return assemble(res.results, B=int(np.asarray(inputs["x"]).shape[0]))

